# revision 29
# baseline (speedup 1.0000x reference)
"""Data-parallel BNN forward kernel for Trainium2 (8 NeuronCores).

Computes (matching the jax reference):
    h  = x @ sign(W1).T + b1          # [B, 100]
    hn = batchnorm(h; batch stats, eps=1e-4) * gamma + beta
    a  = sign(hn)                     # {-1, +1}
    o  = a @ sign(W2).T + b2          # [B, 1000]
    out = log_softmax(o, axis=-1)

Sharding: batch-parallel across 8 cores (4096 rows each), weights
replicated, BN batch statistics combined with one 800-byte AllReduce.

GEMM1 runs as an exact fp16 2-split (x = hi + lo, both fp16; sign
weights are exactly representable), accumulated in fp32 PSUM, which
reproduces fp32 accuracy. x tiles are transposed on the tensor engine
(feature-major is required for the PE's contraction axis).
"""
import numpy as np

B, D, H, O = 32768, 4096, 100, 1000
NCORES = 8
BC = B // NCORES          # batch rows per core
BN_EPS = 1e-4

TB = 512                  # batch tile for GEMM1 (PSUM free dim)
NBT = BC // TB            # 8 batch tiles per core
NKC = D // 128            # 32 feature chunks
FH = 2048                 # feature half loaded per x DMA
KA = H + 2                # GEMM2 contraction with 2 bias rows

_CACHE = {}


def _build_nc(reps=1, variant="full"):
    from concourse import bacc, mybir
    import concourse.tile as tile
    from concourse.masks import make_identity

    f32, f16 = mybir.dt.float32, mybir.dt.float16
    AF = mybir.ActivationFunctionType
    ALU = mybir.AluOpType

    class _Bacc(bacc.Bacc):
        """Bacc whose activation-table pass keeps ONE resident func set.

        The stock pass re-loads a table at every Exp<->Ln switch (66 loads
        per iteration here, each a multi-us ACT stall).  Every activation
        this kernel uses (copy/identity/sign/exp/ln) lives in the single
        act_info.json set 'natural_log_exp_and_others', so remap all loads
        to that set and drop the redundant ones (they carry no semaphores;
        ACT executes in FIFO program order).
        """

        def insert_act_table_loads(self):
            super().insert_act_table_loads()
            from concourse.hw_specs import get_activation_tables
            tables = get_activation_tables(self.m.arch)
            names = list(tables.keys())
            target = names.index("natural_log_exp_and_others")
            allowed = tables["natural_log_exp_and_others"]
            used = {
                i.func
                for b in self.main_func.blocks
                for i in b.instructions
                if isinstance(i, mybir.InstActivation)
            }
            if not used.issubset(allowed):
                return  # fall back to stock behaviour
            for blk in self.main_func.blocks:
                kept = []
                seen = False
                for ins in blk.instructions:
                    if isinstance(ins, mybir.InstLoadActFuncSet):
                        si = ins.sync_info
                        if si is not None and (len(si.on_wait) > 0
                                               or len(si.on_update) > 0):
                            kept.append(ins)  # never drop synced insts
                            continue
                        if seen:
                            continue
                        ins.act_func_set_id = target
                        kept.append(ins)
                        seen = True
                    else:
                        kept.append(ins)
                blk.instructions = kept

    nc = _Bacc(num_devices=NCORES)

    x = nc.dram_tensor("x", [BC, D], f32, kind="ExternalInput")
    W1 = nc.dram_tensor("W1", [H, D], f32, kind="ExternalInput")
    b1 = nc.dram_tensor("b1", [H], f32, kind="ExternalInput")
    gamma = nc.dram_tensor("gamma", [H], f32, kind="ExternalInput")
    beta = nc.dram_tensor("beta", [H], f32, kind="ExternalInput")
    W2 = nc.dram_tensor("W2", [O, H], f32, kind="ExternalInput")
    b2 = nc.dram_tensor("b2", [O], f32, kind="ExternalInput")
    out = nc.dram_tensor("out", [BC, O], f32, kind="ExternalOutput")

    cc_in = nc.dram_tensor("cc_in", [H, 2], f32)
    cc_out = nc.dram_tensor("cc_out", [H, 2], f32, addr_space="Shared")

    with tile.TileContext(nc) as tc:
        with (
            tc.tile_pool(name="const", bufs=1) as cp,
            tc.tile_pool(name="xload", bufs=2) as xp,
            tc.tile_pool(name="hilo", bufs=6) as hp,
            tc.tile_pool(name="wload", bufs=2) as wp,
            tc.tile_pool(name="softmax", bufs=4) as sp,
            tc.tile_pool(name="ps", bufs=2, space="PSUM") as ps,
        ):
          for _rep in range(reps):
            # ---------------- prep: identities, per-channel vectors ----
            ident32 = cp.tile([128, 128], f32)
            make_identity(nc, ident32)
            ident16 = cp.tile([128, 128], f16)
            make_identity(nc, ident16)

            b1_t = cp.tile([H, 1], f32)
            nc.sync.dma_start(out=b1_t, in_=b1[:].unsqueeze(1))
            gamma_t = cp.tile([H, 1], f32)
            nc.sync.dma_start(out=gamma_t, in_=gamma[:].unsqueeze(1))
            beta_t = cp.tile([H, 1], f32)
            nc.sync.dma_start(out=beta_t, in_=beta[:].unsqueeze(1))
            eps_t = cp.tile([H, 1], f32)
            nc.vector.memset(eps_t, BN_EPS)

            # ---------------- prep: sign(W1) transposed chunks ---------
            w1_sb = cp.tile([H, D], f32)
            nc.sync.dma_start(out=w1_sb, in_=W1[:, :])
            sw1n = cp.tile([H, D], f16)
            nc.scalar.activation(out=sw1n, in_=w1_sb, func=AF.Sign)
            sw1t = cp.tile([128, NKC, H], f16)
            for kc in range(NKC):
                pt = ps.tile([128, H], f16, tag="xT", bufs=4)
                nc.tensor.transpose(
                    pt, sw1n[:, kc * 128:(kc + 1) * 128], ident16[:H, :H])
                nc.scalar.copy(out=sw1t[:, kc, :], in_=pt)

            # ---------------- prep: sign(W2).T with bias rows ----------
            sw2aug = cp.tile([KA, O], f16)
            for i in range(8):
                wt = wp.tile([125, H], f32, tag="w2l")
                nc.sync.dma_start(out=wt, in_=W2[i * 125:(i + 1) * 125, :])
                wsg = wp.tile([125, H], f16, tag="w2s")
                nc.scalar.activation(out=wsg, in_=wt, func=AF.Sign)
                pt = ps.tile([H, 125], f16, tag="xT", bufs=4)
                nc.tensor.transpose(pt, wsg, ident16[:125, :125])
                nc.vector.tensor_copy(
                    out=sw2aug[0:H, i * 125:(i + 1) * 125], in_=pt)
            b2_sb = cp.tile([1, O], f32)
            nc.sync.dma_start(out=b2_sb, in_=b2[:].unsqueeze(0))
            b2hi = cp.tile([1, O], f16)
            nc.scalar.copy(out=b2hi, in_=b2_sb)
            b2lo = cp.tile([1, O], f16)
            nc.vector.tensor_tensor(
                out=b2lo, in0=b2_sb, in1=b2hi, op=ALU.subtract)
            nc.sync.dma_start(out=sw2aug[H:H + 1, :], in_=b2hi)
            nc.sync.dma_start(out=sw2aug[H + 1:H + 2, :], in_=b2lo)

            # ---------------- GEMM1: hT[100, BC] = sign(W1) @ x.T + b1 -
            # Software-pipelined: the matmuls for chunk kc are emitted one
            # chunk late, so the PE's FIFO never stalls on the current
            # chunk's ACT(hi-cast)->DVE(lo-sub) chain.
            hT = cp.tile([H, BC], f32)
            for bt in range(NBT):
                h_ps = ps.tile([H, TB], f32, tag="h")
                lag = None  # (kc, hi, lo) waiting for its matmuls

                def emit_mm(h_ps=None):
                    nonlocal lag
                    if lag is None:
                        return
                    kc_, hi_, lo_ = lag
                    nc.tensor.matmul(
                        h_ps, sw1t[:, kc_, :], hi_,
                        start=(kc_ == 0), stop=False)
                    nc.tensor.matmul(
                        h_ps, sw1t[:, kc_, :], lo_,
                        start=False, stop=(kc_ == NKC - 1))
                    lag = None

                for fh in range(D // FH):
                    xb = xp.tile([128, 4, FH], f32, tag="xnat")
                    dma_eng = nc.sync if fh == 0 else nc.scalar
                    dma_eng.dma_start(
                        out=xb,
                        in_=x[bt * TB:(bt + 1) * TB,
                              fh * FH:(fh + 1) * FH].rearrange(
                                  "(s p) f -> p s f", p=128))
                    for k8 in range(FH // 128):
                        kc = fh * (FH // 128) + k8
                        xt_ps = ps.tile([128, TB], f32, tag="xT", bufs=4)
                        for s in range(4):
                            nc.tensor.transpose(
                                xt_ps[:, s * 128:(s + 1) * 128],
                                xb[:, s, k8 * 128:(k8 + 1) * 128],
                                ident32)
                        hi = hp.tile([128, TB], f16, tag="hi")
                        nc.scalar.copy(out=hi, in_=xt_ps)
                        lo = hp.tile([128, TB], f16, tag="lo")
                        nc.vector.tensor_tensor(
                            out=lo, in0=xt_ps, in1=hi, op=ALU.subtract)
                        emit_mm(h_ps)
                        lag = (kc, hi, lo)
                emit_mm(h_ps)
                nc.scalar.activation(
                    out=hT[:, bt * TB:(bt + 1) * TB], in_=h_ps,
                    func=AF.Identity, bias=b1_t)

            # ---------------- BN stats + AllReduce ---------------------
            if variant == "gemm1":
                for t4 in range(BC // 512):
                    res4 = sp.tile([128, 4, O], f32, tag="res4", bufs=2)
                    for ti in range(4):
                        nc.vector.tensor_copy(
                            out=res4[0:H, ti, 0:1000],
                            in_=hT[:, (ti * 1000):(ti * 1000) + 1000])
                    nc.sync.dma_start(
                        out=out[t4 * 512:(t4 + 1) * 512, :].rearrange(
                            "(s p) f -> p s f", p=128),
                        in_=res4)
                continue
            stats = cp.tile([H, NBT, 6], f32)
            for i in range(NBT):
                nc.vector.bn_stats(
                    out=stats[:, i, :], in_=hT[:, i * TB:(i + 1) * TB])
            mv = cp.tile([H, 2], f32)
            nc.vector.bn_aggr(out=mv, in_=stats)
            if variant == "noar":
                mu = mv[:, 0:1]
                varg = mv[:, 1:2]
            else:
                # payload: [mean/8, (var + mean^2)/8]
                msq = cp.tile([H, 1], f32)
                nc.vector.tensor_mul(out=msq, in0=mv[:, 0:1], in1=mv[:, 0:1])
                e2 = cp.tile([H, 1], f32)
                nc.vector.tensor_add(out=e2, in0=mv[:, 1:2], in1=msq)
                ccs = cp.tile([H, 2], f32)
                nc.scalar.mul(out=ccs[:, 0:1], in_=mv[:, 0:1],
                              mul=1.0 / NCORES)
                nc.scalar.mul(out=ccs[:, 1:2], in_=e2, mul=1.0 / NCORES)
                nc.gpsimd.dma_start(out=cc_in[:, :], in_=ccs)
                nc.gpsimd.collective_compute(
                    "AllReduce", ALU.add,
                    replica_groups=[list(range(NCORES))],
                    ins=[cc_in[:, :]], outs=[cc_out[:, :]])
                g = cp.tile([H, 2], f32)
                nc.gpsimd.dma_start(out=g, in_=cc_out[:, :])

                mu = g[:, 0:1]
                musq = cp.tile([H, 1], f32)
                nc.vector.tensor_mul(out=musq, in0=mu, in1=mu)
                varg = cp.tile([H, 1], f32)
                nc.vector.tensor_sub(out=varg, in0=g[:, 1:2], in1=musq)
            lnv = cp.tile([H, 1], f32)
            nc.scalar.activation(out=lnv, in_=varg, func=AF.Ln, bias=eps_t)
            rstd = cp.tile([H, 1], f32)
            nc.scalar.activation(out=rstd, in_=lnv, func=AF.Exp, scale=-0.5)

            # t = (h - mu) * rstd; a = sign(t * gamma + beta)
            # chunked so GEMM2 tiles can start before the whole batch is
            # normalized
            aT = cp.tile([KA, BC], f16)
            nc.vector.memset(aT, 1.0)
            for i in range(NBT):
                sl = slice(i * TB, (i + 1) * TB)
                nc.vector.tensor_scalar(
                    out=hT[:, sl], in0=hT[:, sl], scalar1=mu, scalar2=rstd,
                    op0=ALU.subtract, op1=ALU.mult)
                nc.scalar.activation(
                    out=aT[0:H, sl], in_=hT[:, sl], func=AF.Sign,
                    scale=gamma_t, bias=beta_t)

            # ---------------- GEMM2 + log_softmax ----------------------
            # Also software-pipelined: exp/ln/final-sub for tile t are
            # emitted while tile t+1's matmul/evac/max run, so the ACT
            # FIFO never stalls on the DVE reduce of the current tile.
            NT = BC // 128
            res4_tiles = {}
            slag = None  # (t, o_sb, m, negm)

            def emit_tail():
                nonlocal slag
                if slag is None:
                    return
                t_, o_sb_, m_, negm_ = slag
                e = sp.tile([128, O], f32, tag="e", bufs=2)
                s = sp.tile([128, 1], f32, tag="s")
                nc.scalar.activation(out=e, in_=o_sb_, func=AF.Exp,
                                     bias=negm_, accum_out=s)
                lse = sp.tile([128, 1], f32, tag="lse")
                nc.scalar.activation(out=lse, in_=s, func=AF.Ln)
                c = sp.tile([128, 1], f32, tag="c")
                nc.vector.tensor_add(out=c, in0=m_, in1=lse)
                t4_, ti_ = divmod(t_, 4)
                nc.gpsimd.tensor_scalar(
                    out=res4_tiles[t4_][:, ti_, :], in0=o_sb_, scalar1=c,
                    scalar2=None, op0=ALU.subtract)
                if ti_ == 3:
                    nc.gpsimd.dma_start(
                        out=out[t4_ * 512:(t4_ + 1) * 512, :].rearrange(
                            "(s p) f -> p s f", p=128),
                        in_=res4_tiles.pop(t4_))
                slag = None

            for t in range(NT):
                t4, ti = divmod(t, 4)
                if ti == 0:
                    res4 = sp.tile([128, 4, O], f32, tag="res4", bufs=2)
                    res4_tiles[t4] = res4
                o_ps = ps.tile([128, 2, 512], f32, tag="ops", bufs=1)
                asl = aT[:, t * 128:(t + 1) * 128]
                nc.tensor.matmul(o_ps[:, 0, 0:500], asl,
                                 sw2aug[:, 0:500],
                                 start=True, stop=True)
                nc.tensor.matmul(o_ps[:, 1, 0:500], asl,
                                 sw2aug[:, 500:1000],
                                 start=True, stop=True)
                o_sb = sp.tile([128, O], f32, tag="osb")
                nc.scalar.copy(out=o_sb[:, 0:500], in_=o_ps[:, 0, 0:500])
                nc.scalar.copy(out=o_sb[:, 500:1000],
                               in_=o_ps[:, 1, 0:500])
                m = sp.tile([128, 1], f32, tag="m")
                nc.vector.reduce_max(out=m, in_=o_sb,
                                     axis=mybir.AxisListType.X)
                negm = sp.tile([128, 1], f32, tag="negm")
                nc.vector.tensor_scalar_mul(out=negm, in0=m, scalar1=-1.0)
                emit_tail()
                slag = (t, o_sb, m, negm)
            emit_tail()

    nc.finalize()
    return nc


def _get_runner(reps=1, variant="full"):
    """Compile (once) and return a callable running the SPMD kernel.

    Mirrors bass2jax.run_bass_via_pjrt's multi-core path, but without
    donated output buffers so repeated calls don't re-transfer them, and
    with device-resident input support for timing.
    """
    key = ("runner", reps, variant)
    if key in _CACHE:
        return _CACHE[key]

    import jax
    import jax.numpy as jnp
    from jax.sharding import Mesh, PartitionSpec
    from concourse import mybir
    from concourse import bass2jax
    from concourse.bass2jax import _bass_exec_p, install_neuronx_cc_hook

    try:
        from jax.shard_map import shard_map
    except Exception:
        from jax.experimental.shard_map import shard_map

    install_neuronx_cc_hook()
    nc = _build_nc(reps=reps, variant=variant)

    partition_name = (nc.partition_id_tensor.name
                      if nc.partition_id_tensor else None)
    in_names, out_names, out_avals = [], [], []
    for alloc in nc.m.functions[0].allocations:
        if not isinstance(alloc, mybir.MemoryLocationSet):
            continue
        name = alloc.memorylocations[0].name
        if alloc.kind == "ExternalInput":
            if name != partition_name:
                in_names.append(name)
        elif alloc.kind == "ExternalOutput":
            out_names.append(name)
            shape = tuple(alloc.tensor_shape)
            dtype = mybir.dt.np(alloc.dtype)
            out_avals.append(jax.core.ShapedArray(shape, dtype))
    n_params = len(in_names)
    all_in_names = list(in_names) + list(out_names)
    if partition_name is not None:
        all_in_names.append(partition_name)

    def _body(*args):
        operands = list(args)
        if partition_name is not None:
            operands.append(bass2jax.partition_id_tensor())
        outs = _bass_exec_p.bind(
            *operands,
            out_avals=tuple(out_avals),
            in_names=tuple(all_in_names),
            out_names=tuple(out_names),
            lowering_input_output_aliases=(),
            sim_require_finite=True,
            sim_require_nnan=True,
            nc=nc,
        )
        return tuple(outs)

    devices = jax.devices()[:NCORES]
    mesh = Mesh(np.asarray(devices), ("core",))
    n_outs = len(out_names)
    in_specs = (PartitionSpec("core"),) * (n_params + n_outs)
    out_specs = (PartitionSpec("core"),) * n_outs
    sharded = jax.jit(
        shard_map(_body, mesh=mesh, in_specs=in_specs, out_specs=out_specs,
                  check_rep=False),
        keep_unused=True,
    )
    zeros = [np.zeros((NCORES * a.shape[0], *a.shape[1:]), a.dtype)
             for a in out_avals]
    runner = {
        "sharded": sharded,
        "in_names": in_names,
        "out_names": out_names,
        "zeros": zeros,
        "mesh": mesh,
    }
    _CACHE[key] = runner
    return runner


def _concat_inputs(inputs):
    """Build the global (n_cores*dim0, ...) arrays the shard_map expects."""
    full = {}
    full["x"] = np.ascontiguousarray(inputs["x"], dtype=np.float32)
    for name in ("W1", "b1", "gamma", "beta", "W2", "b2"):
        a = np.ascontiguousarray(inputs[name], dtype=np.float32)
        full[name] = np.concatenate([a] * NCORES, axis=0)
    return full


def run_on_device(inputs, iters=1, reps=1, variant="full"):
    """Run the kernel; returns (full_output, list_of_exec_wall_times_s)."""
    import time
    import jax
    from jax.sharding import NamedSharding, PartitionSpec

    r = _get_runner(reps=reps, variant=variant)
    full = _concat_inputs(inputs)
    shard = NamedSharding(r["mesh"], PartitionSpec("core"))
    dev_args = [jax.device_put(full[n], shard) for n in r["in_names"]]
    dev_zeros = [jax.device_put(z, shard) for z in r["zeros"]]
    # warmup / compile
    outs = r["sharded"](*dev_args, *dev_zeros)
    jax.block_until_ready(outs)
    times = []
    for _ in range(iters):
        t0 = time.perf_counter()
        outs = r["sharded"](*dev_args, *dev_zeros)
        jax.block_until_ready(outs)
        times.append(time.perf_counter() - t0)
    result = np.asarray(outs[r["out_names"].index("out")])
    return result, times


def kernel(**inputs):
    result, _ = run_on_device(inputs, iters=0)
    return result


if __name__ == "__main__":
    # smoke test with small random data is not possible (shapes fixed);
    # run the full thing
    rng = np.random.default_rng(0)
    inputs = {
        "x": rng.standard_normal((B, D), dtype=np.float32),
        "W1": (rng.standard_normal((H, D)) * 0.05).astype(np.float32),
        "b1": (rng.standard_normal(H) * 0.05).astype(np.float32),
        "gamma": np.ones(H, np.float32),
        "beta": np.zeros(H, np.float32),
        "W2": (rng.standard_normal((O, H)) * 0.05).astype(np.float32),
        "b2": (rng.standard_normal(O) * 0.05).astype(np.float32),
    }
    out, times = run_on_device(inputs, iters=3)
    print("out", out.shape, out.dtype)
    print("times:", times)
